# revision 2
# baseline (speedup 1.0000x reference)
"""Trainium2 Bass kernel for a BAN (bilinear attention network) layer — v2.

Per batch b, head h (hd=64; softmax scale + att_w folded into Wqw on host):
    vpT  = (v @ Wv + bv)^T        bf16 [hid=512, V=1024]
    qpwT = (q @ Wqw + bqw)^T      bf16 [hid, Q=512]
    logits_h = vp_h @ qpw_h^T     K=64 MMs, two heads row-packed (0/64)
    e = exp(logits)               ScalarE: 3-bank/2-bank grouped ACTIVATEs
                                  plus some solo ACTIVATEs with accumulator
    s_v = rowsum_q(e)             ACT accumulator (solo) or DVE reduce (groups)
    rb  = 1/(V*s) bf16
    z_h = sum_c rb_c^T @ e_c      M=1 MMs, 4 heads col-packed (0/32/64/96)
    pooled_v_h = ((z_h @ q) @ Wq_h) + bq_h      (contract q with RAW q, then Wq)
    pooled_q_h = (1/Q) cv @ Wv_h + (V/Q) bv_h   (cv = colsum_v v; w rows sum to 1)
    out = relu(fused @ Wo + bo)   via lhsT=fusedT chunks (M=2), Wo row-permuted

Sharding: data-parallel over batch, 2 batches per core, params replicated,
no collectives.  Host does layout transforms / weight folding only.
"""

import numpy as np
import ml_dtypes

BF16 = ml_dtypes.bfloat16

B, V_NUM, Q_NUM = 16, 1024, 512
V_DIM, Q_DIM = 256, 128
HIDDEN, HEADS, HD = 512, 8, 64
SCALE = HD ** -0.5

N_CORES = 8
BPC = B // N_CORES
DC = V_DIM // 128            # 2
IB = HIDDEN // 128           # 4
QC = Q_NUM // 128            # 4
VCH = V_NUM // 128           # 8
NPAIR = HEADS // 2           # 4
KC = 8

# --- tuning knobs ---
# Exp-tile plan per head-pair (16 tiles): tokens A (3-bank ACT group),
# B (2-bank ACT group), S (3 solo ACTIVATEs w/ accumulator in ringA banks).
PAIR_PLANS = [
    ["A", "B", "A", "B", "A", "S"],
    ["A", "B", "S", "A", "B", "S"],
    ["A", "B", "A", "B", "A", "S"],
    ["A", "B", "S", "A", "B", "S"],
    ["A", "B", "A", "B", "A", "S"],
    ["A", "B", "S", "A", "B", "S"],
    ["A", "B", "A", "B", "A", "S"],
    ["A", "B", "A", "B", "A", "S"],
]
DRAINS_PER_BATCH_ON_ACT = 4   # of the 12 projection drains per batch

_CACHE = {}


def _plan_tiles():
    sizes = {"A": 3, "B": 2, "S": 3}
    for p in PAIR_PLANS:
        assert sum(sizes[t] for t in p) == 16
    return sizes


def _build_nc(sim_safe=False):
    from contextlib import ExitStack

    import concourse.tile as tile
    from concourse import bacc, mybir

    f32 = mybir.dt.float32
    bf16 = mybir.dt.bfloat16
    AF = mybir.ActivationFunctionType
    ALU = mybir.AluOpType
    AX = mybir.AxisListType

    sizes = _plan_tiles()
    nc = bacc.Bacc("TRN2", target_bir_lowering=False)

    PCK_VT = 0
    PCK_QT = PCK_VT + BPC * DC * 1024
    PCK_QN = PCK_QT + BPC * 512
    PCK_WALL = PCK_QN + BPC * 4 * 128
    PCK_ID = PCK_WALL + 12 * 512
    PCK_BALL = PCK_ID + 8
    PCK_COLS = PCK_BALL + 2 * 2 * IB
    packed_p = nc.declare_dram_parameter("packed", [128, PCK_COLS], bf16,
                                         isOutput=False)
    rcon_p = nc.declare_dram_parameter("rcon", [1, 10 * 128], bf16,
                                       isOutput=False)
    out_p = nc.declare_dram_parameter("out", [BPC, HIDDEN], f32,
                                      isOutput=True)

    with tile.TileContext(nc) as tc, ExitStack() as ctx:
        const = ctx.enter_context(tc.tile_pool(name="const", bufs=1))
        work = ctx.enter_context(tc.tile_pool(name="work", bufs=1))
        ps_a = ctx.enter_context(tc.tile_pool(name="ps_a", bufs=1, space="PSUM"))
        ps_b = ctx.enter_context(tc.tile_pool(name="ps_b", bufs=1, space="PSUM"))
        ps_z = ctx.enter_context(tc.tile_pool(name="ps_z", bufs=1, space="PSUM"))
        ps_p = ctx.enter_context(tc.tile_pool(name="ps_p", bufs=1, space="PSUM"))
        ps_s = ctx.enter_context(tc.tile_pool(name="ps_s", bufs=1, space="PSUM"))

        packed_sb = const.tile([128, PCK_COLS], bf16, tag="packed")
        nc.sync.dma_start(packed_sb[:], packed_p[:])
        rcon_sb = const.tile([1, 10 * 128], bf16, tag="rcon")
        nc.sync.dma_start(rcon_sb[:], rcon_p[:])

        vt_sb = packed_sb[:, PCK_VT:PCK_QT].rearrange(
            "p (b c v) -> p b c v", b=BPC, c=DC)
        qt_sb = packed_sb[:, PCK_QT:PCK_QN].rearrange(
            "p (b q) -> p b q", b=BPC)
        qn_sb = packed_sb[:, PCK_QN:PCK_WALL].rearrange(
            "p (b c d) -> p b c d", b=BPC, c=QC)
        wall_sb = packed_sb[:, PCK_WALL:PCK_ID].rearrange(
            "p (w h) -> p w h", w=12)
        wv_sb = wall_sb[:, 0:DC]
        wqw_sb = wall_sb[:, DC]
        wq_sb = wall_sb[:, DC + 1]
        wo_sb = wall_sb[:, DC + 2:DC + 10]
        ident_sb = packed_sb[:, PCK_ID:PCK_ID + 8]
        ball_sb = packed_sb[:, PCK_BALL:PCK_COLS].bitcast(f32)
        bv_sb = ball_sb[:, 0:IB]
        bqw_sb = ball_sb[:, IB:2 * IB]
        rcon = rcon_sb.rearrange("p (k d) -> p k d", k=10)
        # rcon rows: 0..3 = [bq_2j | bq_2j+1]; 4..7 = (V/Q)*bv ib-block;
        # 8 = ones; 9 = bo (cols 0:128 unused; bo passed via boT below)
        one_sb = rcon[0:1, 8, 0:1]
        boT_p = nc.declare_dram_parameter("boT", [1, HIDDEN], bf16,
                                          isOutput=False)
        boT_sb = const.tile([1, HIDDEN], bf16, tag="boT")
        nc.sync.dma_start(boT_sb[:], boT_p[:])

        vpT_sb = work.tile([128, BPC, IB, 1024], bf16, tag="vpt")
        qpwT_sb = work.tile([128, BPC, IB, Q_NUM], bf16, tag="qpwt")
        e_sb = work.tile([128, 3, 16, 512], bf16, tag="e")
        s_sb = work.tile([128, BPC, NPAIR, 16], f32, tag="s")
        rb_sb = work.tile([128, BPC, NPAIR, 16], bf16, tag="rb")
        rbf_sb = work.tile([128, 16], f32, tag="rbf")
        zscr_sb = work.tile([128, 2, 512], bf16, tag="zscr")
        zrows_sb = work.tile([36, BPC, 512], bf16, tag="zrows")
        zT_sb = work.tile([128, BPC, QC, 8], bf16, tag="zT")
        zqT_sb = work.tile([128, BPC, 8], bf16, tag="zqT")
        fusedT_sb = work.tile([128, KC, BPC], bf16, tag="fused")
        cv_sb = work.tile([128, BPC, DC], f32, tag="cv")
        cvb_sb = work.tile([128, BPC, DC], bf16, tag="cvb")
        out_sb = work.tile([BPC, HIDDEN], f32, tag="out")

        # ---------- prologue (projections, cv, pq) ----------
        drain_ct = [0]

        def drain(dst, src, bias):
            i = drain_ct[0] % 12
            drain_ct[0] += 1
            if i < DRAINS_PER_BATCH_ON_ACT:
                nc.scalar.activation(dst, src, AF.Identity, bias=bias)
            else:
                nc.vector.tensor_scalar_add(dst, src, bias)

        def prologue_thunks(b):
            th = []
            for ib in range(IB):
                def qpw_fill(ib=ib):
                    ps = ps_p.tile([128, 512], f32, tag="proj",
                                   name=f"qpp_{b}_{ib}")
                    nc.tensor.matmul(
                        ps[:], lhsT=wqw_sb[:, ib * 128:(ib + 1) * 128],
                        rhs=qt_sb[:, b, :], start=True, stop=True)
                    drain(qpwT_sb[:, b, ib, :], ps[:], bqw_sb[:, ib:ib + 1])
                for vb in range(2):
                    def vpt_fill(ib=ib, vb=vb):
                        ps = ps_p.tile([128, 512], f32, tag="proj",
                                       name=f"vpp_{b}_{ib}_{vb}")
                        for dc in range(DC):
                            nc.tensor.matmul(
                                ps[:],
                                lhsT=wv_sb[:, dc, ib * 128:(ib + 1) * 128],
                                rhs=vt_sb[:, b, dc, vb * 512:(vb + 1) * 512],
                                start=(dc == 0), stop=(dc == DC - 1))
                        drain(vpT_sb[:, b, ib, vb * 512:(vb + 1) * 512],
                              ps[:], bv_sb[:, ib:ib + 1])
                    th.append(vpt_fill)
                th.append(qpw_fill)

            def cv_fill():
                nc.vector.tensor_reduce(
                    cv_sb[:, b, :], vt_sb[:, b], axis=AX.X, op=ALU.add)
                nc.vector.tensor_scalar_mul(
                    cvb_sb[:, b, :], cv_sb[:, b, :], 1.0 / Q_NUM)
            th.append(cv_fill)

            for ib in range(IB):
                def pq_fill(ib=ib):
                    ps = ps_s.tile([128, 16], f32, tag="sm",
                                   name=f"pqs_{b}_{ib}")
                    for dc in range(DC):
                        nc.tensor.matmul(
                            ps[:, 0:1],
                            lhsT=wv_sb[:, dc, ib * 128:(ib + 1) * 128],
                            rhs=cvb_sb[:, b, dc:dc + 1],
                            start=(dc == 0), stop=False)
                    nc.tensor.matmul(
                        ps[:, 0:1], lhsT=rcon[:, 4 + ib, :], rhs=one_sb,
                        start=False, stop=True)
                    nc.vector.tensor_copy(fusedT_sb[:, 4 + ib, b:b + 1],
                                          ps[:, 0:1])
                th.append(pq_fill)
            return th

        # ---------- exp pairs ----------
        def emit_pair(b, t, filler):
            er = (b * NPAIR + t) % 3

            def pop2():
                for _ in range(2):
                    if filler:
                        filler.pop(0)()

            idx = 0
            pend = None

            def do_exps(tok, tiles, ring):
                t0, n = tiles[0], len(tiles)
                if tok == "S":
                    for j, tidx in enumerate(tiles):
                        nc.scalar.activation(
                            e_sb[:, er, tidx, :],
                            ring[:, j * 512:(j + 1) * 512], AF.Exp,
                            accum_out=s_sb[:, b, t, tidx:tidx + 1])
                else:
                    nc.scalar.activation(
                        e_sb[:, er, t0:t0 + n].rearrange("p c q -> p (c q)"),
                        ring[:, 0:n * 512], AF.Exp)
                    with nc.allow_low_precision(reason="f32-accum rowsum"):
                        nc.vector.tensor_reduce(
                            s_sb[:, b, t, t0:t0 + n],
                            e_sb[:, er, t0:t0 + n], axis=AX.X, op=ALU.add)

            for tok in PAIR_PLANS[b * NPAIR + t]:
                n = sizes[tok]
                tiles = list(range(idx, idx + n))
                idx += n
                if tok in ("A", "S"):
                    ring = ps_a.tile([128, 1536], f32, tag="ringA",
                                     name=f"rA_{b}_{t}_{idx}")
                else:
                    ring = ps_b.tile([128, 1024], f32, tag="ringB",
                                     name=f"rB_{b}_{t}_{idx}")
                for j, tidx in enumerate(tiles):
                    c, side = tidx // 2, tidx % 2
                    hb = 64 * side
                    nc.tensor.matmul(
                        ring[:, j * 512:(j + 1) * 512],
                        lhsT=vpT_sb[hb:hb + 64, b, t, c * 128:(c + 1) * 128],
                        rhs=qpwT_sb[hb:hb + 64, b, t, :],
                        start=True, stop=True)
                if pend is not None:
                    do_exps(*pend)
                    pop2()
                pend = (tok, tiles, ring)
            do_exps(*pend)
            pop2()
            nc.vector.reciprocal(rbf_sb[:], s_sb[:, b, t, :])
            nc.vector.tensor_scalar_mul(rb_sb[:, b, t, :], rbf_sb[:],
                                        1.0 / V_NUM)

        # ---------- z quads ----------
        def z_quad_thunks(b, tlo):
            th = []
            heads = [(tlo + dt, 2 * (tlo + dt) + side, side)
                     for dt in range(2) for side in range(2)]
            zq = ps_z.tile([128, 512], f32, tag="zq")

            def zinit():
                if not sim_safe:
                    return
                # zero the whole bank (K=1 MM) so the later [97,512] drain
                # copy reads initialized memory; z MMs then accumulate.
                nc.tensor.matmul(
                    zq[:], lhsT=rcon[:, 9, :], rhs=rcon_sb[:, 0:512],
                    start=True, stop=False, skip_group_check=True)
            th.append(zinit)
            for c in range(VCH):
                def zmm(c=c):
                    for j, (t, h, side) in enumerate(heads):
                        er = (b * NPAIR + t) % 3
                        tidx = 2 * c + side
                        nc.tensor.matmul(
                            zq[32 * j:32 * j + 1, :],
                            lhsT=rb_sb[:, b, t, tidx:tidx + 1],
                            rhs=e_sb[:, er, tidx, :],
                            start=(c == 0 and not sim_safe),
                            stop=(c == VCH - 1 and not sim_safe),
                            tile_position=(0, 32 * j),
                            skip_group_check=True)
                th.append(zmm)

            def zfini():
                if not sim_safe:
                    return
                nc.tensor.matmul(
                    zq[:], lhsT=rcon[:, 9, :], rhs=rcon_sb[:, 0:512],
                    start=False, stop=True, skip_group_check=True)
            th.append(zfini)

            def zdrain():
                qd = tlo // 2
                sc = zscr_sb[:, qd, :]
                nc.vector.tensor_copy(sc[0:97, :], zq[0:97, :])
                for j, (t, h, side) in enumerate(heads):
                    nc.sync.dma_start(
                        zrows_sb[32 * qd + (h - 2 * tlo):32 * qd +
                                 (h - 2 * tlo) + 1, b, :],
                        sc[32 * j:32 * j + 1, :])
            th.append(zdrain)
            return th

        # ---------- z tail (per quad qd: heads 4qd..4qd+3) ----------
        def ztail_quad_thunks(b, qd):
            th = []
            for qc in range(QC):
                def ztr(qc=qc):
                    pst = ps_s.tile([128, 16], f32, tag="sm",
                                    name=f"tr_{b}_{qd}_{qc}").bitcast(bf16)
                    nc.tensor.transpose(
                        pst[:, 0:4],
                        zrows_sb[32 * qd:32 * qd + 4, b,
                                 qc * 128:(qc + 1) * 128],
                        ident_sb[32 * qd:32 * qd + 4, 0:4])
                    nc.vector.tensor_copy(zT_sb[:, b, qc, 4 * qd:4 * qd + 4],
                                          pst[:, 0:4])
                th.append(ztr)

            def zqt():
                ps = ps_s.tile([128, 16], f32, tag="sm",
                               name=f"zqts_{b}_{qd}")
                for qc in range(QC):
                    nc.tensor.matmul(
                        ps[:, 0:4], lhsT=qn_sb[:, b, qc, :],
                        rhs=zT_sb[:, b, qc, 4 * qd:4 * qd + 4],
                        start=(qc == 0), stop=(qc == QC - 1))
                nc.vector.tensor_copy(zqT_sb[:, b, 4 * qd:4 * qd + 4],
                                      ps[:, 0:4])
            th.append(zqt)

            for j in (2 * qd, 2 * qd + 1):
                def pv_fill(j=j):
                    h0, h1 = 2 * j, 2 * j + 1
                    ps = ps_s.tile([128, 16], f32, tag="sm",
                                   name=f"pvs_{b}_{j}")
                    nc.tensor.matmul(
                        ps[:, 0:1], lhsT=rcon[:, j, :], rhs=one_sb,
                        start=True, stop=False, skip_group_check=True)
                    nc.tensor.matmul(
                        ps[0:64, 0:1],
                        lhsT=wq_sb[:, h0 * 64:(h0 + 1) * 64],
                        rhs=zqT_sb[:, b, h0:h0 + 1], start=False, stop=False,
                        tile_position=(0, 0), skip_group_check=True)
                    nc.tensor.matmul(
                        ps[64:128, 0:1],
                        lhsT=wq_sb[:, h1 * 64:(h1 + 1) * 64],
                        rhs=zqT_sb[:, b, h1:h1 + 1], start=False, stop=False,
                        tile_position=(0, 64), skip_group_check=True)
                    nc.tensor.matmul(
                        ps[:, 0:1], lhsT=rcon[:, 9, :], rhs=one_sb,
                        start=False, stop=True, skip_group_check=True)
                    nc.vector.tensor_copy(fusedT_sb[:, j, b:b + 1],
                                          ps[:, 0:1])
                th.append(pv_fill)
            return th

        def epilogue():
            ps = ps_p.tile([128, 512], f32, tag="proj", name="epi")
            for kc in range(KC):
                nc.tensor.matmul(
                    ps[0:BPC, :], lhsT=fusedT_sb[:, kc, :],
                    rhs=wo_sb[:, kc], start=(kc == 0), stop=False)
            nc.tensor.matmul(ps[0:BPC, :], lhsT=rcon[:, 8, 0:BPC],
                             rhs=boT_sb[:], start=False, stop=True)
            nc.scalar.activation(out_sb[:], ps[0:BPC, :], AF.Relu)
            nc.sync.dma_start(out_p[:], out_sb[:])

        # ---------- schedule ----------
        pro0 = prologue_thunks(0)
        for fn in pro0[:3]:
            fn()
        filler = list(pro0[3:])
        for b in range(BPC):
            for t in range(NPAIR):
                if b == 0 and t == 0:
                    filler += prologue_thunks(1)
                if t == 2:
                    filler += z_quad_thunks(b, 0)
                if t == 3:
                    filler += ztail_quad_thunks(b, 0)
                if b == 1 and t == 0:
                    filler += z_quad_thunks(0, 2)
                if b == 1 and t == 1:
                    filler += ztail_quad_thunks(0, 1)
                emit_pair(b, t, filler)
        filler += z_quad_thunks(1, 2) + ztail_quad_thunks(1, 1)
        while filler:
            filler.pop(0)()
        epilogue()

    nc.compile()
    return nc


def _get_nc(sim_safe=False):
    key = ("nc", sim_safe)
    if key not in _CACHE:
        _CACHE[key] = _build_nc(sim_safe)
    return _CACHE[key]


def _host_prep(v, q, Wv, bv, Wq, bq, att_w, Wo, bo):
    v = np.asarray(v, np.float32)
    q = np.asarray(q, np.float32)
    Wv = np.asarray(Wv, np.float32)
    bv = np.asarray(bv, np.float32)
    Wq = np.asarray(Wq, np.float32)
    bq = np.asarray(bq, np.float32)
    att_w = np.asarray(att_w, np.float32)
    Wo = np.asarray(Wo, np.float32)
    bo = np.asarray(bo, np.float32)

    Wq_h = Wq.reshape(Q_DIM, HEADS, HD)
    Wqw = (SCALE * np.einsum("dhj,hij->dhi", Wq_h, att_w)).reshape(
        Q_DIM, HIDDEN)
    bqw = (SCALE * np.einsum("hj,hij->hi", bq.reshape(HEADS, HD),
                             att_w)).reshape(HIDDEN)

    # Wo row permutation to match fusedT layout
    perm = np.empty(2 * HIDDEN, np.int64)
    for kc in range(KC):
        for p in range(128):
            h = 2 * (kc % 4) + p // 64
            d = p % 64
            if kc < 4:
                forig = h * 128 + d
            else:
                forig = h * 128 + 64 + d
            perm[kc * 128 + p] = forig
    WoP = Wo[perm]

    wall = np.concatenate([
        Wv.reshape(DC, 128, HIDDEN).transpose(1, 0, 2),
        Wqw.reshape(1, 128, HIDDEN).transpose(1, 0, 2),
        Wq.reshape(1, 128, HIDDEN).transpose(1, 0, 2),
        WoP.reshape(KC, 128, HIDDEN).transpose(1, 0, 2),
    ], axis=1).reshape(128, 12 * HIDDEN)
    ident = np.zeros((128, 8), np.float32)
    ident[:8, :8] = np.eye(8)
    ident[32:36, 0:4] = np.eye(4)
    ball = np.concatenate([bv.reshape(IB, 128).T, bqw.reshape(IB, 128).T],
                          axis=1).astype(np.float32)
    shared_cols = np.concatenate([
        wall.astype(BF16), ident.astype(BF16),
        np.ascontiguousarray(ball).view(BF16)], axis=1)

    # rcon rows
    rcon = np.zeros((10, 128), np.float32)
    bq_h = bq.reshape(HEADS, HD)
    bv_h = bv.reshape(IB, 128)
    for j in range(4):
        rcon[j] = np.concatenate([bq_h[2 * j], bq_h[2 * j + 1]])
    for ib in range(IB):
        rcon[4 + ib] = (V_NUM / Q_NUM) * bv_h[ib]
    rcon[8] = 1.0
    rcon_row = rcon.reshape(1, -1).astype(BF16)
    boT = bo.reshape(1, HIDDEN).astype(BF16)

    in_maps = []
    for i in range(N_CORES):
        sl = slice(i * BPC, (i + 1) * BPC)
        vt = v[sl].transpose(0, 2, 1).reshape(BPC, DC, 128, 1024)
        vt = vt.transpose(2, 0, 1, 3).reshape(128, BPC * DC * 1024)
        qt = q[sl].transpose(0, 2, 1).transpose(1, 0, 2).reshape(
            128, BPC * Q_NUM)
        qn = q[sl].reshape(BPC, QC, 128, Q_DIM).transpose(2, 0, 1, 3).reshape(
            128, BPC * QC * Q_DIM)
        packed = np.concatenate(
            [vt.astype(BF16), qt.astype(BF16), qn.astype(BF16), shared_cols],
            axis=1)
        in_maps.append({"packed": np.ascontiguousarray(packed),
                        "rcon": rcon_row, "boT": boT})
    return in_maps


def kernel(**inputs):
    from concourse.bass_utils import run_bass_kernel_spmd

    nc = _get_nc()
    in_maps = _host_prep(**inputs)
    res = run_bass_kernel_spmd(nc, in_maps, core_ids=list(range(N_CORES)))
    out = np.empty((B, HIDDEN), np.float32)
    for i in range(N_CORES):
        out[i * BPC:(i + 1) * BPC] = np.asarray(res.results[i]["out"])
    return out


# revision 3
# speedup vs baseline: 1.0427x; 1.0427x over previous
"""Trainium2 Bass kernel for a BAN (bilinear attention network) layer — v2.

Per batch b, head h (hd=64; softmax scale + att_w folded into Wqw on host):
    vpT  = (v @ Wv + bv)^T        bf16 [hid=512, V=1024]
    qpwT = (q @ Wqw + bqw)^T      bf16 [hid, Q=512]
    logits_h = vp_h @ qpw_h^T     K=64 MMs, two heads row-packed (0/64)
    e = exp(logits)               ScalarE: 3-bank/2-bank grouped ACTIVATEs
                                  plus some solo ACTIVATEs with accumulator
    s_v = rowsum_q(e)             ACT accumulator (solo) or DVE reduce (groups)
    rb  = 1/(V*s) bf16
    z_h = sum_c rb_c^T @ e_c      M=1 MMs, 4 heads col-packed (0/32/64/96)
    pooled_v_h = ((z_h @ q) @ Wq_h) + bq_h      (contract q with RAW q, then Wq)
    pooled_q_h = (1/Q) cv @ Wv_h + (V/Q) bv_h   (cv = colsum_v v; w rows sum to 1)
    out = relu(fused @ Wo + bo)   via lhsT=fusedT chunks (M=2), Wo row-permuted

Sharding: data-parallel over batch, 2 batches per core, params replicated,
no collectives.  Host does layout transforms / weight folding only.
"""

import numpy as np
import ml_dtypes

BF16 = ml_dtypes.bfloat16

B, V_NUM, Q_NUM = 16, 1024, 512
V_DIM, Q_DIM = 256, 128
HIDDEN, HEADS, HD = 512, 8, 64
SCALE = HD ** -0.5

N_CORES = 8
BPC = B // N_CORES
DC = V_DIM // 128            # 2
IB = HIDDEN // 128           # 4
QC = Q_NUM // 128            # 4
VCH = V_NUM // 128           # 8
NPAIR = HEADS // 2           # 4
KC = 8

# --- tuning knobs ---
# Exp-tile plan per head-pair (16 tiles): tokens A (3-bank ACT group),
# B (2-bank ACT group), S (3 solo ACTIVATEs w/ accumulator in ringA banks).
PAIR_PLANS = [
    ["A", "B", "A", "B", "A", "S"],
    ["A", "B", "S", "A", "B", "S"],
    ["A", "B", "A", "B", "A", "S"],
    ["A", "B", "S", "A", "B", "S"],
    ["A", "B", "A", "B", "A", "S"],
    ["A", "B", "S", "A", "B", "S"],
    ["A", "B", "A", "B", "A", "S"],
    ["A", "B", "A", "B", "A", "S"],
]
DRAINS_PER_BATCH_ON_ACT = 4   # of the 12 projection drains per batch

_CACHE = {}


def _plan_tiles():
    sizes = {"A": 3, "B": 2, "S": 3}
    for p in PAIR_PLANS:
        assert sum(sizes[t] for t in p) == 16
    return sizes


def _build_nc(sim_safe=False):
    from contextlib import ExitStack

    import concourse.tile as tile
    from concourse import bacc, mybir

    f32 = mybir.dt.float32
    bf16 = mybir.dt.bfloat16
    AF = mybir.ActivationFunctionType
    ALU = mybir.AluOpType
    AX = mybir.AxisListType

    sizes = _plan_tiles()
    nc = bacc.Bacc("TRN2", target_bir_lowering=False)

    PCK_VT = 0
    PCK_QT = PCK_VT + BPC * DC * 1024
    PCK_QN = PCK_QT + BPC * 512
    PCK_WALL = PCK_QN + BPC * 4 * 128
    PCK_ID = PCK_WALL + 12 * 512
    PCK_BALL = PCK_ID + 8
    PCK_COLS = PCK_BALL + 2 * 2 * IB
    packed_p = nc.declare_dram_parameter("packed", [128, PCK_COLS], bf16,
                                         isOutput=False)
    rcon_p = nc.declare_dram_parameter("rcon", [1, 10 * 128], bf16,
                                       isOutput=False)
    out_p = nc.declare_dram_parameter("out", [BPC, HIDDEN], f32,
                                      isOutput=True)

    with tile.TileContext(nc) as tc, ExitStack() as ctx:
        const = ctx.enter_context(tc.tile_pool(name="const", bufs=1))
        work = ctx.enter_context(tc.tile_pool(name="work", bufs=1))
        ps_a = ctx.enter_context(tc.tile_pool(name="ps_a", bufs=1, space="PSUM"))
        ps_b = ctx.enter_context(tc.tile_pool(name="ps_b", bufs=1, space="PSUM"))
        ps_z = ctx.enter_context(tc.tile_pool(name="ps_z", bufs=1, space="PSUM"))
        ps_p = ctx.enter_context(tc.tile_pool(name="ps_p", bufs=1, space="PSUM"))
        ps_s = ctx.enter_context(tc.tile_pool(name="ps_s", bufs=1, space="PSUM"))

        packed_sb = const.tile([128, PCK_COLS], bf16, tag="packed")
        nc.sync.dma_start(packed_sb[:], packed_p[:])
        rcon_sb = const.tile([1, 10 * 128], bf16, tag="rcon")
        nc.sync.dma_start(rcon_sb[:], rcon_p[:])

        vt_sb = packed_sb[:, PCK_VT:PCK_QT].rearrange(
            "p (b c v) -> p b c v", b=BPC, c=DC)
        qt_sb = packed_sb[:, PCK_QT:PCK_QN].rearrange(
            "p (b q) -> p b q", b=BPC)
        qn_sb = packed_sb[:, PCK_QN:PCK_WALL].rearrange(
            "p (b c d) -> p b c d", b=BPC, c=QC)
        wall_sb = packed_sb[:, PCK_WALL:PCK_ID].rearrange(
            "p (w h) -> p w h", w=12)
        wv_sb = wall_sb[:, 0:DC]
        wqw_sb = wall_sb[:, DC]
        wq_sb = wall_sb[:, DC + 1]
        wo_sb = wall_sb[:, DC + 2:DC + 10]
        ident_sb = packed_sb[:, PCK_ID:PCK_ID + 8]
        ball_sb = packed_sb[:, PCK_BALL:PCK_COLS].bitcast(f32)
        bv_sb = ball_sb[:, 0:IB]
        bqw_sb = ball_sb[:, IB:2 * IB]
        rcon = rcon_sb.rearrange("p (k d) -> p k d", k=10)
        # rcon rows: 0..3 = [bq_2j | bq_2j+1]; 4..7 = (V/Q)*bv ib-block;
        # 8 = ones; 9 = bo (cols 0:128 unused; bo passed via boT below)
        one_sb = rcon[0:1, 8, 0:1]
        boT_p = nc.declare_dram_parameter("boT", [1, HIDDEN], bf16,
                                          isOutput=False)
        boT_sb = const.tile([1, HIDDEN], bf16, tag="boT")
        nc.sync.dma_start(boT_sb[:], boT_p[:])

        vpT_sb = work.tile([128, BPC, IB, 1024], bf16, tag="vpt")
        qpwT_sb = work.tile([128, BPC, IB, Q_NUM], bf16, tag="qpwt")
        e_sb = work.tile([128, 3, 16, 512], bf16, tag="e")
        s_sb = work.tile([128, BPC, NPAIR, 16], f32, tag="s")
        rb_sb = work.tile([128, BPC, NPAIR, 16], bf16, tag="rb")
        rbf_sb = work.tile([128, 16], f32, tag="rbf")
        zscr_sb = work.tile([128, 2, 512], bf16, tag="zscr")
        zrows_sb = work.tile([36, BPC, 512], bf16, tag="zrows")
        zT_sb = work.tile([128, BPC, QC, 8], bf16, tag="zT")
        zqT_sb = work.tile([128, BPC, 8], bf16, tag="zqT")
        fusedT_sb = work.tile([128, KC, BPC], bf16, tag="fused")
        cv_sb = work.tile([128, BPC, DC], f32, tag="cv")
        cvb_sb = work.tile([128, BPC, DC], bf16, tag="cvb")
        out_sb = work.tile([BPC, HIDDEN], f32, tag="out")

        # ---------- prologue (projections, cv, pq) ----------
        drain_ct = [0]

        def drain(dst, src, bias):
            i = drain_ct[0] % 12
            drain_ct[0] += 1
            if i < DRAINS_PER_BATCH_ON_ACT:
                nc.scalar.activation(dst, src, AF.Identity, bias=bias)
            else:
                nc.vector.tensor_scalar_add(dst, src, bias)

        def prologue_thunks(b, use_rings=False):
            th = []
            rot = [0]

            def proj_ps(name):
                if not use_rings:
                    return ps_p.tile([128, 512], f32, tag="proj", name=name)
                r = rot[0] % 6
                rot[0] += 1
                if r < 3:
                    if r == 0:
                        rot.append(ps_a.tile([128, 1536], f32, tag="ringA",
                                             name=name + "_ra"))
                    return rot[-1][:, r * 512:(r + 1) * 512]
                if r == 3:
                    rot.append(ps_b.tile([128, 1024], f32, tag="ringB",
                                         name=name + "_rb"))
                    return rot[-1][:, 0:512]
                if r == 4:
                    return rot[-1][:, 512:1024]
                return ps_p.tile([128, 512], f32, tag="proj", name=name)
            for ib in range(IB):
                def qpw_fill(ib=ib):
                    ps = proj_ps(f"qpp_{b}_{ib}")
                    nc.tensor.matmul(
                        ps[:], lhsT=wqw_sb[:, ib * 128:(ib + 1) * 128],
                        rhs=qt_sb[:, b, :], start=True, stop=True)
                    drain(qpwT_sb[:, b, ib, :], ps[:], bqw_sb[:, ib:ib + 1])
                for vb in range(2):
                    def vpt_fill(ib=ib, vb=vb):
                        ps = proj_ps(f"vpp_{b}_{ib}_{vb}")
                        for dc in range(DC):
                            nc.tensor.matmul(
                                ps[:],
                                lhsT=wv_sb[:, dc, ib * 128:(ib + 1) * 128],
                                rhs=vt_sb[:, b, dc, vb * 512:(vb + 1) * 512],
                                start=(dc == 0), stop=(dc == DC - 1))
                        drain(vpT_sb[:, b, ib, vb * 512:(vb + 1) * 512],
                              ps[:], bv_sb[:, ib:ib + 1])
                    th.append(vpt_fill)
                th.append(qpw_fill)

            def cv_fill():
                nc.vector.tensor_reduce(
                    cv_sb[:, b, :], vt_sb[:, b], axis=AX.X, op=ALU.add)
                nc.vector.tensor_scalar_mul(
                    cvb_sb[:, b, :], cv_sb[:, b, :], 1.0 / Q_NUM)
            th.append(cv_fill)

            for ib in range(IB):
                def pq_fill(ib=ib):
                    ps = ps_s.tile([128, 16], f32, tag="sm",
                                   name=f"pqs_{b}_{ib}")
                    for dc in range(DC):
                        nc.tensor.matmul(
                            ps[:, 0:1],
                            lhsT=wv_sb[:, dc, ib * 128:(ib + 1) * 128],
                            rhs=cvb_sb[:, b, dc:dc + 1],
                            start=(dc == 0), stop=False)
                    nc.tensor.matmul(
                        ps[:, 0:1], lhsT=rcon[:, 4 + ib, :], rhs=one_sb,
                        start=False, stop=True)
                    nc.vector.tensor_copy(fusedT_sb[:, 4 + ib, b:b + 1],
                                          ps[:, 0:1])
                th.append(pq_fill)
            return th

        # ---------- exp pairs ----------
        def emit_pair(b, t, filler):
            er = (b * NPAIR + t) % 3

            def pop2():
                for _ in range(2):
                    if filler:
                        filler.pop(0)()

            idx = 0
            pend = None

            def do_exps(tok, tiles, ring):
                t0, n = tiles[0], len(tiles)
                if tok == "S":
                    for j, tidx in enumerate(tiles):
                        nc.scalar.activation(
                            e_sb[:, er, tidx, :],
                            ring[:, j * 512:(j + 1) * 512], AF.Exp,
                            accum_out=s_sb[:, b, t, tidx:tidx + 1])
                else:
                    nc.scalar.activation(
                        e_sb[:, er, t0:t0 + n].rearrange("p c q -> p (c q)"),
                        ring[:, 0:n * 512], AF.Exp)
                    with nc.allow_low_precision(reason="f32-accum rowsum"):
                        nc.vector.tensor_reduce(
                            s_sb[:, b, t, t0:t0 + n],
                            e_sb[:, er, t0:t0 + n], axis=AX.X, op=ALU.add)

            for tok in PAIR_PLANS[b * NPAIR + t]:
                n = sizes[tok]
                tiles = list(range(idx, idx + n))
                idx += n
                if tok in ("A", "S"):
                    ring = ps_a.tile([128, 1536], f32, tag="ringA",
                                     name=f"rA_{b}_{t}_{idx}")
                else:
                    ring = ps_b.tile([128, 1024], f32, tag="ringB",
                                     name=f"rB_{b}_{t}_{idx}")
                for j, tidx in enumerate(tiles):
                    c, side = tidx // 2, tidx % 2
                    hb = 64 * side
                    nc.tensor.matmul(
                        ring[:, j * 512:(j + 1) * 512],
                        lhsT=vpT_sb[hb:hb + 64, b, t, c * 128:(c + 1) * 128],
                        rhs=qpwT_sb[hb:hb + 64, b, t, :],
                        start=True, stop=True)
                if pend is not None:
                    do_exps(*pend)
                    pop2()
                pend = (tok, tiles, ring)
            do_exps(*pend)
            pop2()
            nc.vector.reciprocal(rbf_sb[:], s_sb[:, b, t, :])
            nc.vector.tensor_scalar_mul(rb_sb[:, b, t, :], rbf_sb[:],
                                        1.0 / V_NUM)

        # ---------- z quads ----------
        def z_quad_thunks(b, tlo):
            th = []
            heads = [(tlo + dt, 2 * (tlo + dt) + side, side)
                     for dt in range(2) for side in range(2)]
            zq = ps_z.tile([128, 512], f32, tag="zq")

            def zinit():
                if not sim_safe:
                    return
                # zero the whole bank (K=1 MM) so the later [97,512] drain
                # copy reads initialized memory; z MMs then accumulate.
                nc.tensor.matmul(
                    zq[:], lhsT=rcon[:, 9, :], rhs=rcon_sb[:, 0:512],
                    start=True, stop=False, skip_group_check=True)
            th.append(zinit)
            for c in range(VCH):
                def zmm(c=c):
                    for j, (t, h, side) in enumerate(heads):
                        er = (b * NPAIR + t) % 3
                        tidx = 2 * c + side
                        nc.tensor.matmul(
                            zq[32 * j:32 * j + 1, :],
                            lhsT=rb_sb[:, b, t, tidx:tidx + 1],
                            rhs=e_sb[:, er, tidx, :],
                            start=(c == 0 and not sim_safe),
                            stop=(c == VCH - 1 and not sim_safe),
                            tile_position=(0, 32 * j),
                            skip_group_check=True)
                th.append(zmm)

            def zfini():
                if not sim_safe:
                    return
                nc.tensor.matmul(
                    zq[:], lhsT=rcon[:, 9, :], rhs=rcon_sb[:, 0:512],
                    start=False, stop=True, skip_group_check=True)
            th.append(zfini)

            def zdrain():
                qd = tlo // 2
                sc = zscr_sb[:, qd, :]
                nc.vector.tensor_copy(sc[0:97, :], zq[0:97, :])
                for j, (t, h, side) in enumerate(heads):
                    nc.sync.dma_start(
                        zrows_sb[32 * qd + (h - 2 * tlo):32 * qd +
                                 (h - 2 * tlo) + 1, b, :],
                        sc[32 * j:32 * j + 1, :])
            th.append(zdrain)
            return th

        # ---------- z tail (per quad qd: heads 4qd..4qd+3) ----------
        def ztail_quad_thunks(b, qd):
            th = []
            for qc in range(QC):
                def ztr(qc=qc):
                    pst = ps_s.tile([128, 16], f32, tag="sm",
                                    name=f"tr_{b}_{qd}_{qc}").bitcast(bf16)
                    nc.tensor.transpose(
                        pst[:, 0:4],
                        zrows_sb[32 * qd:32 * qd + 4, b,
                                 qc * 128:(qc + 1) * 128],
                        ident_sb[32 * qd:32 * qd + 4, 0:4])
                    nc.vector.tensor_copy(zT_sb[:, b, qc, 4 * qd:4 * qd + 4],
                                          pst[:, 0:4])
                th.append(ztr)

            def zqt():
                ps = ps_s.tile([128, 16], f32, tag="sm",
                               name=f"zqts_{b}_{qd}")
                for qc in range(QC):
                    nc.tensor.matmul(
                        ps[:, 0:4], lhsT=qn_sb[:, b, qc, :],
                        rhs=zT_sb[:, b, qc, 4 * qd:4 * qd + 4],
                        start=(qc == 0), stop=(qc == QC - 1))
                nc.vector.tensor_copy(zqT_sb[:, b, 4 * qd:4 * qd + 4],
                                      ps[:, 0:4])
            th.append(zqt)

            for j in (2 * qd, 2 * qd + 1):
                def pv_fill(j=j):
                    h0, h1 = 2 * j, 2 * j + 1
                    ps = ps_s.tile([128, 16], f32, tag="sm",
                                   name=f"pvs_{b}_{j}")
                    nc.tensor.matmul(
                        ps[:, 0:1], lhsT=rcon[:, j, :], rhs=one_sb,
                        start=True, stop=False, skip_group_check=True)
                    nc.tensor.matmul(
                        ps[0:64, 0:1],
                        lhsT=wq_sb[:, h0 * 64:(h0 + 1) * 64],
                        rhs=zqT_sb[:, b, h0:h0 + 1], start=False, stop=False,
                        tile_position=(0, 0), skip_group_check=True)
                    nc.tensor.matmul(
                        ps[64:128, 0:1],
                        lhsT=wq_sb[:, h1 * 64:(h1 + 1) * 64],
                        rhs=zqT_sb[:, b, h1:h1 + 1], start=False, stop=False,
                        tile_position=(0, 64), skip_group_check=True)
                    nc.tensor.matmul(
                        ps[:, 0:1], lhsT=rcon[:, 9, :], rhs=one_sb,
                        start=False, stop=True, skip_group_check=True)
                    nc.vector.tensor_copy(fusedT_sb[:, j, b:b + 1],
                                          ps[:, 0:1])
                th.append(pv_fill)
            return th

        def epilogue():
            ps = ps_p.tile([128, 512], f32, tag="proj", name="epi")
            for kc in range(KC):
                nc.tensor.matmul(
                    ps[0:BPC, :], lhsT=fusedT_sb[:, kc, :],
                    rhs=wo_sb[:, kc], start=(kc == 0), stop=False)
            nc.tensor.matmul(ps[0:BPC, :], lhsT=rcon[:, 8, 0:BPC],
                             rhs=boT_sb[:], start=False, stop=True)
            nc.scalar.activation(out_sb[:], ps[0:BPC, :], AF.Relu)
            nc.sync.dma_start(out_p[:], out_sb[:])

        # ---------- schedule ----------
        pro0 = prologue_thunks(0)
        for fn in pro0[:3]:
            fn()
        filler = list(pro0[3:])
        for b in range(BPC):
            for t in range(NPAIR):
                if b == 0 and t == 0:
                    filler += prologue_thunks(1)
                if t == 2:
                    filler += z_quad_thunks(b, 0)
                if t == 3:
                    filler += ztail_quad_thunks(b, 0)
                if b == 1 and t == 0:
                    filler += z_quad_thunks(0, 2)
                if b == 1 and t == 1:
                    filler += ztail_quad_thunks(0, 1)
                emit_pair(b, t, filler)
        filler += z_quad_thunks(1, 2) + ztail_quad_thunks(1, 1)
        while filler:
            filler.pop(0)()
        epilogue()

    nc.compile()
    return nc


def _get_nc(sim_safe=False):
    key = ("nc", sim_safe)
    if key not in _CACHE:
        _CACHE[key] = _build_nc(sim_safe)
    return _CACHE[key]


def _host_prep(v, q, Wv, bv, Wq, bq, att_w, Wo, bo):
    v = np.asarray(v, np.float32)
    q = np.asarray(q, np.float32)
    Wv = np.asarray(Wv, np.float32)
    bv = np.asarray(bv, np.float32)
    Wq = np.asarray(Wq, np.float32)
    bq = np.asarray(bq, np.float32)
    att_w = np.asarray(att_w, np.float32)
    Wo = np.asarray(Wo, np.float32)
    bo = np.asarray(bo, np.float32)

    Wq_h = Wq.reshape(Q_DIM, HEADS, HD)
    Wqw = (SCALE * np.einsum("dhj,hij->dhi", Wq_h, att_w)).reshape(
        Q_DIM, HIDDEN)
    bqw = (SCALE * np.einsum("hj,hij->hi", bq.reshape(HEADS, HD),
                             att_w)).reshape(HIDDEN)

    # Wo row permutation to match fusedT layout
    perm = np.empty(2 * HIDDEN, np.int64)
    for kc in range(KC):
        for p in range(128):
            h = 2 * (kc % 4) + p // 64
            d = p % 64
            if kc < 4:
                forig = h * 128 + d
            else:
                forig = h * 128 + 64 + d
            perm[kc * 128 + p] = forig
    WoP = Wo[perm]

    wall = np.concatenate([
        Wv.reshape(DC, 128, HIDDEN).transpose(1, 0, 2),
        Wqw.reshape(1, 128, HIDDEN).transpose(1, 0, 2),
        Wq.reshape(1, 128, HIDDEN).transpose(1, 0, 2),
        WoP.reshape(KC, 128, HIDDEN).transpose(1, 0, 2),
    ], axis=1).reshape(128, 12 * HIDDEN)
    ident = np.zeros((128, 8), np.float32)
    ident[:8, :8] = np.eye(8)
    ident[32:36, 0:4] = np.eye(4)
    ball = np.concatenate([bv.reshape(IB, 128).T, bqw.reshape(IB, 128).T],
                          axis=1).astype(np.float32)
    shared_cols = np.concatenate([
        wall.astype(BF16), ident.astype(BF16),
        np.ascontiguousarray(ball).view(BF16)], axis=1)

    # rcon rows
    rcon = np.zeros((10, 128), np.float32)
    bq_h = bq.reshape(HEADS, HD)
    bv_h = bv.reshape(IB, 128)
    for j in range(4):
        rcon[j] = np.concatenate([bq_h[2 * j], bq_h[2 * j + 1]])
    for ib in range(IB):
        rcon[4 + ib] = (V_NUM / Q_NUM) * bv_h[ib]
    rcon[8] = 1.0
    rcon_row = rcon.reshape(1, -1).astype(BF16)
    boT = bo.reshape(1, HIDDEN).astype(BF16)

    in_maps = []
    for i in range(N_CORES):
        sl = slice(i * BPC, (i + 1) * BPC)
        vt = v[sl].transpose(0, 2, 1).reshape(BPC, DC, 128, 1024)
        vt = vt.transpose(2, 0, 1, 3).reshape(128, BPC * DC * 1024)
        qt = q[sl].transpose(0, 2, 1).transpose(1, 0, 2).reshape(
            128, BPC * Q_NUM)
        qn = q[sl].reshape(BPC, QC, 128, Q_DIM).transpose(2, 0, 1, 3).reshape(
            128, BPC * QC * Q_DIM)
        packed = np.concatenate(
            [vt.astype(BF16), qt.astype(BF16), qn.astype(BF16), shared_cols],
            axis=1)
        in_maps.append({"packed": np.ascontiguousarray(packed),
                        "rcon": rcon_row, "boT": boT})
    return in_maps


def kernel(**inputs):
    from concourse.bass_utils import run_bass_kernel_spmd

    nc = _get_nc()
    in_maps = _host_prep(**inputs)
    res = run_bass_kernel_spmd(nc, in_maps, core_ids=list(range(N_CORES)))
    out = np.empty((B, HIDDEN), np.float32)
    for i in range(N_CORES):
        out[i * BPC:(i + 1) * BPC] = np.asarray(res.results[i]["out"])
    return out


# revision 4
# speedup vs baseline: 1.0888x; 1.0442x over previous
"""Trainium2 Bass kernel for a BAN (bilinear attention network) layer — v2.

Per batch b, head h (hd=64; softmax scale + att_w folded into Wqw on host):
    vpT  = (v @ Wv + bv)^T        bf16 [hid=512, V=1024]
    qpwT = (q @ Wqw + bqw)^T      bf16 [hid, Q=512]
    logits_h = vp_h @ qpw_h^T     K=64 MMs, two heads row-packed (0/64)
    e = exp(logits)               ScalarE: 3-bank/2-bank grouped ACTIVATEs
                                  plus some solo ACTIVATEs with accumulator
    s_v = rowsum_q(e)             ACT accumulator (solo) or DVE reduce (groups)
    rb  = 1/(V*s) bf16
    z_h = sum_c rb_c^T @ e_c      M=1 MMs, 4 heads col-packed (0/32/64/96)
    pooled_v_h = ((z_h @ q) @ Wq_h) + bq_h      (contract q with RAW q, then Wq)
    pooled_q_h = (1/Q) cv @ Wv_h + (V/Q) bv_h   (cv = colsum_v v; w rows sum to 1)
    out = relu(fused @ Wo + bo)   via lhsT=fusedT chunks (M=2), Wo row-permuted

Sharding: data-parallel over batch, 2 batches per core, params replicated,
no collectives.  Host does layout transforms / weight folding only.
"""

import numpy as np
import ml_dtypes

BF16 = ml_dtypes.bfloat16

B, V_NUM, Q_NUM = 16, 1024, 512
V_DIM, Q_DIM = 256, 128
HIDDEN, HEADS, HD = 512, 8, 64
SCALE = HD ** -0.5

N_CORES = 8
BPC = B // N_CORES
DC = V_DIM // 128            # 2
IB = HIDDEN // 128           # 4
QC = Q_NUM // 128            # 4
VCH = V_NUM // 128           # 8
NPAIR = HEADS // 2           # 4
KC = 8

# --- tuning knobs ---
# Exp-tile plan per head-pair (16 tiles): tokens A (3-bank ACT group),
# B (2-bank ACT group), S (3 solo ACTIVATEs w/ accumulator in ringA banks).
PAIR_PLANS = [
    ["A", "B", "A", "B", "A", "S"],
    ["A", "B", "S", "A", "B", "S"],
    ["A", "B", "A", "B", "A", "S"],
    ["A", "B", "S", "A", "B", "S"],
    ["A", "B", "A", "B", "A", "S"],
    ["A", "B", "S", "A", "B", "S"],
    ["A", "B", "A", "B", "A", "S"],
    ["A", "B", "A", "B", "A", "S"],
]
DRAINS_PER_BATCH_ON_ACT = 4   # of the 12 projection drains per batch

_CACHE = {}


def _plan_tiles():
    sizes = {"A": 3, "B": 2, "S": 3}
    for p in PAIR_PLANS:
        assert sum(sizes[t] for t in p) == 16
    return sizes


def _build_nc(sim_safe=False):
    from contextlib import ExitStack

    import concourse.tile as tile
    from concourse import bacc, mybir

    f32 = mybir.dt.float32
    bf16 = mybir.dt.bfloat16
    AF = mybir.ActivationFunctionType
    ALU = mybir.AluOpType
    AX = mybir.AxisListType

    sizes = _plan_tiles()
    nc = bacc.Bacc("TRN2", target_bir_lowering=False)

    PCK_VT = 0
    PCK_QT = PCK_VT + BPC * DC * 1024
    PCK_QN = PCK_QT + BPC * 512
    PCK_WALL = PCK_QN + BPC * 4 * 128
    PCK_ID = PCK_WALL + 12 * 512
    PCK_BALL = PCK_ID + 8
    PCK_COLS = PCK_BALL + 2 * 2 * IB
    packed_p = nc.declare_dram_parameter("packed", [128, PCK_COLS], bf16,
                                         isOutput=False)
    rcon_p = nc.declare_dram_parameter("rcon", [1, 10 * 128], bf16,
                                       isOutput=False)
    out_p = nc.declare_dram_parameter("out", [BPC, HIDDEN], f32,
                                      isOutput=True)

    with tile.TileContext(nc) as tc, ExitStack() as ctx:
        const = ctx.enter_context(tc.tile_pool(name="const", bufs=1))
        work = ctx.enter_context(tc.tile_pool(name="work", bufs=1))
        ps_a = ctx.enter_context(tc.tile_pool(name="ps_a", bufs=1, space="PSUM"))
        ps_b = ctx.enter_context(tc.tile_pool(name="ps_b", bufs=1, space="PSUM"))
        ps_z = ctx.enter_context(tc.tile_pool(name="ps_z", bufs=1, space="PSUM"))
        ps_p = ctx.enter_context(tc.tile_pool(name="ps_p", bufs=1, space="PSUM"))
        ps_s = ctx.enter_context(tc.tile_pool(name="ps_s", bufs=1, space="PSUM"))

        packed_sb = const.tile([128, PCK_COLS], bf16, tag="packed")
        nc.sync.dma_start(packed_sb[:], packed_p[:])
        rcon_sb = const.tile([1, 10 * 128], bf16, tag="rcon")
        nc.sync.dma_start(rcon_sb[:], rcon_p[:])

        vt_sb = packed_sb[:, PCK_VT:PCK_QT].rearrange(
            "p (b c v) -> p b c v", b=BPC, c=DC)
        qt_sb = packed_sb[:, PCK_QT:PCK_QN].rearrange(
            "p (b q) -> p b q", b=BPC)
        qn_sb = packed_sb[:, PCK_QN:PCK_WALL].rearrange(
            "p (b c d) -> p b c d", b=BPC, c=QC)
        wall_sb = packed_sb[:, PCK_WALL:PCK_ID].rearrange(
            "p (w h) -> p w h", w=12)
        wv_sb = wall_sb[:, 0:DC]
        wqw_sb = wall_sb[:, DC]
        wq_sb = wall_sb[:, DC + 1]
        wo_sb = wall_sb[:, DC + 2:DC + 10]
        ident_sb = packed_sb[:, PCK_ID:PCK_ID + 8]
        ball_sb = packed_sb[:, PCK_BALL:PCK_COLS].bitcast(f32)
        bv_sb = ball_sb[:, 0:IB]
        bqw_sb = ball_sb[:, IB:2 * IB]
        rcon = rcon_sb.rearrange("p (k d) -> p k d", k=10)
        # rcon rows: 0..3 = [bq_2j | bq_2j+1]; 4..7 = (V/Q)*bv ib-block;
        # 8 = ones; 9 = bo (cols 0:128 unused; bo passed via boT below)
        one_sb = rcon[0:1, 8, 0:1]
        boT_p = nc.declare_dram_parameter("boT", [1, HIDDEN], bf16,
                                          isOutput=False)
        boT_sb = const.tile([1, HIDDEN], bf16, tag="boT")
        nc.sync.dma_start(boT_sb[:], boT_p[:])

        vpT_sb = work.tile([128, BPC, IB, 1024], bf16, tag="vpt")
        qpwT_sb = work.tile([128, BPC, IB, Q_NUM], bf16, tag="qpwt")
        e_sb = work.tile([128, 3, 16, 512], bf16, tag="e")
        s_sb = work.tile([128, BPC, NPAIR, 16], f32, tag="s")
        rb_sb = work.tile([128, BPC, NPAIR, 16], bf16, tag="rb")
        rbf_sb = work.tile([128, 16], f32, tag="rbf")
        zscr_sb = work.tile([128, 2, 512], bf16, tag="zscr")
        zrows_sb = work.tile([36, BPC, 512], bf16, tag="zrows")
        zT_sb = work.tile([128, BPC, QC, 8], bf16, tag="zT")
        zqT_sb = work.tile([128, BPC, 8], bf16, tag="zqT")
        fusedT_sb = work.tile([128, KC, BPC], bf16, tag="fused")
        cv_sb = work.tile([128, BPC, DC], f32, tag="cv")
        cvb_sb = work.tile([128, BPC, DC], bf16, tag="cvb")
        out_sb = work.tile([BPC, HIDDEN], f32, tag="out")

        # ---------- prologue (projections, cv, pq) ----------
        drain_ct = [0]

        def drain(dst, src, bias):
            i = drain_ct[0] % 12
            drain_ct[0] += 1
            if i < DRAINS_PER_BATCH_ON_ACT:
                nc.scalar.activation(dst, src, AF.Identity, bias=bias)
            else:
                nc.vector.tensor_scalar_add(dst, src, bias)

        def prologue_thunks(b, use_rings=False):
            th = []
            rot = [0]

            def proj_ps(name):
                r = rot[0]
                rot[0] += 1
                if r % 2 == 1:
                    return ps_z.tile([128, 512], f32, tag="zq",
                                     name=name + "_z")
                return ps_p.tile([128, 512], f32, tag="proj", name=name)
            def _unused(name):
                if not use_rings:
                    return ps_p.tile([128, 512], f32, tag="proj", name=name)
                r = rot[0] % 6
                rot[0] += 1
                if r < 3:
                    if r == 0:
                        rot.append(ps_a.tile([128, 1536], f32, tag="ringA",
                                             name=name + "_ra"))
                    return rot[-1][:, r * 512:(r + 1) * 512]
                if r == 3:
                    rot.append(ps_b.tile([128, 1024], f32, tag="ringB",
                                         name=name + "_rb"))
                    return rot[-1][:, 0:512]
                if r == 4:
                    return rot[-1][:, 512:1024]
                return ps_p.tile([128, 512], f32, tag="proj", name=name)
            for ib in range(IB):
                def qpw_fill(ib=ib):
                    ps = proj_ps(f"qpp_{b}_{ib}")
                    nc.tensor.matmul(
                        ps[:], lhsT=wqw_sb[:, ib * 128:(ib + 1) * 128],
                        rhs=qt_sb[:, b, :], start=True, stop=True)
                    drain(qpwT_sb[:, b, ib, :], ps[:], bqw_sb[:, ib:ib + 1])
                for vb in range(2):
                    def vpt_fill(ib=ib, vb=vb):
                        ps = proj_ps(f"vpp_{b}_{ib}_{vb}")
                        for dc in range(DC):
                            nc.tensor.matmul(
                                ps[:],
                                lhsT=wv_sb[:, dc, ib * 128:(ib + 1) * 128],
                                rhs=vt_sb[:, b, dc, vb * 512:(vb + 1) * 512],
                                start=(dc == 0), stop=(dc == DC - 1))
                        drain(vpT_sb[:, b, ib, vb * 512:(vb + 1) * 512],
                              ps[:], bv_sb[:, ib:ib + 1])
                    th.append(vpt_fill)
                th.append(qpw_fill)

            def cv_fill():
                nc.vector.tensor_reduce(
                    cv_sb[:, b, :], vt_sb[:, b], axis=AX.X, op=ALU.add)
                nc.vector.tensor_scalar_mul(
                    cvb_sb[:, b, :], cv_sb[:, b, :], 1.0 / Q_NUM)
            th.append(cv_fill)

            for ib in range(IB):
                def pq_fill(ib=ib):
                    ps = ps_s.tile([128, 16], f32, tag="sm",
                                   name=f"pqs_{b}_{ib}")
                    for dc in range(DC):
                        nc.tensor.matmul(
                            ps[:, 0:1],
                            lhsT=wv_sb[:, dc, ib * 128:(ib + 1) * 128],
                            rhs=cvb_sb[:, b, dc:dc + 1],
                            start=(dc == 0), stop=False)
                    nc.tensor.matmul(
                        ps[:, 0:1], lhsT=rcon[:, 4 + ib, :], rhs=one_sb,
                        start=False, stop=True)
                    nc.vector.tensor_copy(fusedT_sb[:, 4 + ib, b:b + 1],
                                          ps[:, 0:1])
                th.append(pq_fill)
            return th

        # ---------- exp pairs ----------
        def emit_pair(b, t, filler):
            er = (b * NPAIR + t) % 3

            def pop2():
                for _ in range(2):
                    if filler:
                        filler.pop(0)()

            idx = 0
            pend = None

            def do_exps(tok, tiles, ring):
                t0, n = tiles[0], len(tiles)
                if tok == "S":
                    for j, tidx in enumerate(tiles):
                        nc.scalar.activation(
                            e_sb[:, er, tidx, :],
                            ring[:, j * 512:(j + 1) * 512], AF.Exp,
                            accum_out=s_sb[:, b, t, tidx:tidx + 1])
                else:
                    nc.scalar.activation(
                        e_sb[:, er, t0:t0 + n].rearrange("p c q -> p (c q)"),
                        ring[:, 0:n * 512], AF.Exp)
                    with nc.allow_low_precision(reason="f32-accum rowsum"):
                        nc.vector.tensor_reduce(
                            s_sb[:, b, t, t0:t0 + n],
                            e_sb[:, er, t0:t0 + n], axis=AX.X, op=ALU.add)

            for tok in PAIR_PLANS[b * NPAIR + t]:
                n = sizes[tok]
                tiles = list(range(idx, idx + n))
                idx += n
                if tok in ("A", "S"):
                    ring = ps_a.tile([128, 1536], f32, tag="ringA",
                                     name=f"rA_{b}_{t}_{idx}")
                else:
                    ring = ps_b.tile([128, 1024], f32, tag="ringB",
                                     name=f"rB_{b}_{t}_{idx}")
                for j, tidx in enumerate(tiles):
                    c, side = tidx // 2, tidx % 2
                    hb = 64 * side
                    nc.tensor.matmul(
                        ring[:, j * 512:(j + 1) * 512],
                        lhsT=vpT_sb[hb:hb + 64, b, t, c * 128:(c + 1) * 128],
                        rhs=qpwT_sb[hb:hb + 64, b, t, :],
                        start=True, stop=True)
                if pend is not None:
                    do_exps(*pend)
                    pop2()
                pend = (tok, tiles, ring)
            do_exps(*pend)
            pop2()
            nc.vector.reciprocal(rbf_sb[:], s_sb[:, b, t, :])
            nc.vector.tensor_scalar_mul(rb_sb[:, b, t, :], rbf_sb[:],
                                        1.0 / V_NUM)

        # ---------- z quads ----------
        def z_quad_thunks(b, tlo):
            th = []
            heads = [(tlo + dt, 2 * (tlo + dt) + side, side)
                     for dt in range(2) for side in range(2)]
            zq = ps_z.tile([128, 512], f32, tag="zq")

            def zinit():
                if not sim_safe:
                    return
                # zero the whole bank (K=1 MM) so the later [97,512] drain
                # copy reads initialized memory; z MMs then accumulate.
                nc.tensor.matmul(
                    zq[:], lhsT=rcon[:, 9, :], rhs=rcon_sb[:, 0:512],
                    start=True, stop=False, skip_group_check=True)
            th.append(zinit)
            for c in range(VCH):
                def zmm(c=c):
                    for j, (t, h, side) in enumerate(heads):
                        er = (b * NPAIR + t) % 3
                        tidx = 2 * c + side
                        nc.tensor.matmul(
                            zq[32 * j:32 * j + 1, :],
                            lhsT=rb_sb[:, b, t, tidx:tidx + 1],
                            rhs=e_sb[:, er, tidx, :],
                            start=(c == 0 and not sim_safe),
                            stop=(c == VCH - 1 and not sim_safe),
                            tile_position=(0, 32 * j),
                            skip_group_check=True)
                th.append(zmm)

            def zfini():
                if not sim_safe:
                    return
                nc.tensor.matmul(
                    zq[:], lhsT=rcon[:, 9, :], rhs=rcon_sb[:, 0:512],
                    start=False, stop=True, skip_group_check=True)
            th.append(zfini)

            def zdrain():
                qd = tlo // 2
                sc = zscr_sb[:, qd, :]
                nc.vector.tensor_copy(sc[0:97, :], zq[0:97, :])
                for j, (t, h, side) in enumerate(heads):
                    nc.sync.dma_start(
                        zrows_sb[32 * qd + (h - 2 * tlo):32 * qd +
                                 (h - 2 * tlo) + 1, b, :],
                        sc[32 * j:32 * j + 1, :])
            th.append(zdrain)
            return th

        # ---------- z tail (per quad qd: heads 4qd..4qd+3) ----------
        def ztail_quad_thunks(b, qd):
            th = []
            for qc in range(QC):
                def ztr(qc=qc):
                    pst = ps_s.tile([128, 16], f32, tag="sm",
                                    name=f"tr_{b}_{qd}_{qc}").bitcast(bf16)
                    nc.tensor.transpose(
                        pst[:, 0:4],
                        zrows_sb[32 * qd:32 * qd + 4, b,
                                 qc * 128:(qc + 1) * 128],
                        ident_sb[32 * qd:32 * qd + 4, 0:4])
                    nc.vector.tensor_copy(zT_sb[:, b, qc, 4 * qd:4 * qd + 4],
                                          pst[:, 0:4])
                th.append(ztr)

            def zqt():
                ps = ps_s.tile([128, 16], f32, tag="sm",
                               name=f"zqts_{b}_{qd}")
                for qc in range(QC):
                    nc.tensor.matmul(
                        ps[:, 0:4], lhsT=qn_sb[:, b, qc, :],
                        rhs=zT_sb[:, b, qc, 4 * qd:4 * qd + 4],
                        start=(qc == 0), stop=(qc == QC - 1))
                nc.vector.tensor_copy(zqT_sb[:, b, 4 * qd:4 * qd + 4],
                                      ps[:, 0:4])
            th.append(zqt)

            for j in (2 * qd, 2 * qd + 1):
                def pv_fill(j=j):
                    h0, h1 = 2 * j, 2 * j + 1
                    ps = ps_s.tile([128, 16], f32, tag="sm",
                                   name=f"pvs_{b}_{j}")
                    nc.tensor.matmul(
                        ps[:, 0:1], lhsT=rcon[:, j, :], rhs=one_sb,
                        start=True, stop=False, skip_group_check=True)
                    nc.tensor.matmul(
                        ps[0:64, 0:1],
                        lhsT=wq_sb[:, h0 * 64:(h0 + 1) * 64],
                        rhs=zqT_sb[:, b, h0:h0 + 1], start=False, stop=False,
                        tile_position=(0, 0), skip_group_check=True)
                    nc.tensor.matmul(
                        ps[64:128, 0:1],
                        lhsT=wq_sb[:, h1 * 64:(h1 + 1) * 64],
                        rhs=zqT_sb[:, b, h1:h1 + 1], start=False, stop=False,
                        tile_position=(0, 64), skip_group_check=True)
                    nc.tensor.matmul(
                        ps[:, 0:1], lhsT=rcon[:, 9, :], rhs=one_sb,
                        start=False, stop=True, skip_group_check=True)
                    nc.vector.tensor_copy(fusedT_sb[:, j, b:b + 1],
                                          ps[:, 0:1])
                th.append(pv_fill)
            return th

        def epilogue():
            ps = ps_p.tile([128, 512], f32, tag="proj", name="epi")
            for kc in range(KC):
                nc.tensor.matmul(
                    ps[0:BPC, :], lhsT=fusedT_sb[:, kc, :],
                    rhs=wo_sb[:, kc], start=(kc == 0), stop=False)
            nc.tensor.matmul(ps[0:BPC, :], lhsT=rcon[:, 8, 0:BPC],
                             rhs=boT_sb[:], start=False, stop=True)
            nc.scalar.activation(out_sb[:], ps[0:BPC, :], AF.Relu)
            nc.sync.dma_start(out_p[:], out_sb[:])

        # ---------- schedule ----------
        pro0 = prologue_thunks(0)
        for fn in pro0[:3]:
            fn()
        filler = list(pro0[3:])
        for b in range(BPC):
            for t in range(NPAIR):
                if b == 0 and t == 0:
                    filler += prologue_thunks(1)
                if t == 2:
                    filler += z_quad_thunks(b, 0)
                if t == 3:
                    filler += ztail_quad_thunks(b, 0)
                if b == 1 and t == 0:
                    filler += z_quad_thunks(0, 2)
                if b == 1 and t == 1:
                    filler += ztail_quad_thunks(0, 1)
                emit_pair(b, t, filler)
        filler += z_quad_thunks(1, 2) + ztail_quad_thunks(1, 1)
        while filler:
            filler.pop(0)()
        epilogue()

    nc.compile()
    return nc


def _get_nc(sim_safe=False):
    key = ("nc", sim_safe)
    if key not in _CACHE:
        _CACHE[key] = _build_nc(sim_safe)
    return _CACHE[key]


def _host_prep(v, q, Wv, bv, Wq, bq, att_w, Wo, bo):
    v = np.asarray(v, np.float32)
    q = np.asarray(q, np.float32)
    Wv = np.asarray(Wv, np.float32)
    bv = np.asarray(bv, np.float32)
    Wq = np.asarray(Wq, np.float32)
    bq = np.asarray(bq, np.float32)
    att_w = np.asarray(att_w, np.float32)
    Wo = np.asarray(Wo, np.float32)
    bo = np.asarray(bo, np.float32)

    Wq_h = Wq.reshape(Q_DIM, HEADS, HD)
    Wqw = (SCALE * np.einsum("dhj,hij->dhi", Wq_h, att_w)).reshape(
        Q_DIM, HIDDEN)
    bqw = (SCALE * np.einsum("hj,hij->hi", bq.reshape(HEADS, HD),
                             att_w)).reshape(HIDDEN)

    # Wo row permutation to match fusedT layout
    perm = np.empty(2 * HIDDEN, np.int64)
    for kc in range(KC):
        for p in range(128):
            h = 2 * (kc % 4) + p // 64
            d = p % 64
            if kc < 4:
                forig = h * 128 + d
            else:
                forig = h * 128 + 64 + d
            perm[kc * 128 + p] = forig
    WoP = Wo[perm]

    wall = np.concatenate([
        Wv.reshape(DC, 128, HIDDEN).transpose(1, 0, 2),
        Wqw.reshape(1, 128, HIDDEN).transpose(1, 0, 2),
        Wq.reshape(1, 128, HIDDEN).transpose(1, 0, 2),
        WoP.reshape(KC, 128, HIDDEN).transpose(1, 0, 2),
    ], axis=1).reshape(128, 12 * HIDDEN)
    ident = np.zeros((128, 8), np.float32)
    ident[:8, :8] = np.eye(8)
    ident[32:36, 0:4] = np.eye(4)
    ball = np.concatenate([bv.reshape(IB, 128).T, bqw.reshape(IB, 128).T],
                          axis=1).astype(np.float32)
    shared_cols = np.concatenate([
        wall.astype(BF16), ident.astype(BF16),
        np.ascontiguousarray(ball).view(BF16)], axis=1)

    # rcon rows
    rcon = np.zeros((10, 128), np.float32)
    bq_h = bq.reshape(HEADS, HD)
    bv_h = bv.reshape(IB, 128)
    for j in range(4):
        rcon[j] = np.concatenate([bq_h[2 * j], bq_h[2 * j + 1]])
    for ib in range(IB):
        rcon[4 + ib] = (V_NUM / Q_NUM) * bv_h[ib]
    rcon[8] = 1.0
    rcon_row = rcon.reshape(1, -1).astype(BF16)
    boT = bo.reshape(1, HIDDEN).astype(BF16)

    in_maps = []
    for i in range(N_CORES):
        sl = slice(i * BPC, (i + 1) * BPC)
        vt = v[sl].transpose(0, 2, 1).reshape(BPC, DC, 128, 1024)
        vt = vt.transpose(2, 0, 1, 3).reshape(128, BPC * DC * 1024)
        qt = q[sl].transpose(0, 2, 1).transpose(1, 0, 2).reshape(
            128, BPC * Q_NUM)
        qn = q[sl].reshape(BPC, QC, 128, Q_DIM).transpose(2, 0, 1, 3).reshape(
            128, BPC * QC * Q_DIM)
        packed = np.concatenate(
            [vt.astype(BF16), qt.astype(BF16), qn.astype(BF16), shared_cols],
            axis=1)
        in_maps.append({"packed": np.ascontiguousarray(packed),
                        "rcon": rcon_row, "boT": boT})
    return in_maps


def kernel(**inputs):
    from concourse.bass_utils import run_bass_kernel_spmd

    nc = _get_nc()
    in_maps = _host_prep(**inputs)
    res = run_bass_kernel_spmd(nc, in_maps, core_ids=list(range(N_CORES)))
    out = np.empty((B, HIDDEN), np.float32)
    for i in range(N_CORES):
        out[i * BPC:(i + 1) * BPC] = np.asarray(res.results[i]["out"])
    return out
